# revision 24
# baseline (speedup 1.0000x reference)
"""Trainium2 Bass kernel for nn_CrossAttention (B=4, N=M=2048, DIM=1024, H=16, D=64).

Sharding: batch x head-group over 8 cores. Core c handles batch b = c//2 and
heads hgrp = c%2 (8 heads, a contiguous 512-wide slice of the hidden dim).
Each core computes q/k/v projections for its heads, flash-style attention in
S^T layout (keys on partitions), and a *partial* output projection over its
512 hidden dims. The host sums the two partials per batch and adds the output
bias (the only cross-core reduction).

Device layouts (chosen so every matmul has its contraction dim on partitions):
  xT/cT   [E=1024, N=2048]   (host-transposed, bf16)
  qT      [512, 2048]        d-on-partitions, produced by lhsT=W^T, rhs=xT
  kT_z    [128, DC, 2, M]    zero-padded per head-of-pair: slot hp holds that
                             head's 64 d-rows, the other 64 rows are 0.  Every
                             QK matmul then uses a full 128-row stationary so
                             ALL matmuls share one (128,128) PE tile config —
                             mixed 64/128 configs cost ~90ns per transition.
  v_aug   [m, head, 65]      keys-on-partitions; col 64 == 1.0 so the PV
                             matmul also yields the softmax denominator
  S^T     [m=128, n=1024]    PSUM; exp on ScalarE (scale=1/8 fused)
  O^T     [65, 512] PSUM     partition 64 = sum_m exp(S); normalization via a
                             K=1 broadcast matmul + reciprocal + multiply
"""

import numpy as np
import ml_dtypes

import concourse.bass as bass
import concourse.mybir as mybir
import concourse.tile as tile
from concourse import bacc
from concourse.bass_utils import run_bass_kernel_spmd

N_CORES = 8
B, N, M, DIM = 4, 2048, 2048, 1024
HEADS, HD = 16, 64            # total heads, head dim
HPC = 8                       # heads per core
CW = HPC * HD                 # per-core hidden width = 512
EC = DIM // 128               # 8 contraction chunks of 128
DC = CW // 128                # 4 chunks of the per-core q/k dims
MC = M // 128                 # 16 key chunks
NB = N // 512                 # 4 query 512-blocks
SCALE = HD ** -0.5            # 0.125

F32 = mybir.dt.float32
BF16 = mybir.dt.bfloat16
nbf = ml_dtypes.bfloat16


def _build_nc():
    nc = bacc.Bacc("TRN2", target_bir_lowering=False, debug=False,
                   num_devices=N_CORES)

    xT = nc.dram_tensor("xT", [DIM, N], BF16, kind="ExternalInput")
    cT = nc.dram_tensor("cT", [DIM, M], BF16, kind="ExternalInput")
    wq = nc.dram_tensor("wq", [DIM, CW], BF16, kind="ExternalInput")
    wk = nc.dram_tensor("wk", [DIM, CW], BF16, kind="ExternalInput")
    wv = nc.dram_tensor("wv", [DIM, CW], BF16, kind="ExternalInput")
    wo = nc.dram_tensor("wo", [CW, DIM], BF16, kind="ExternalInput")
    bq = nc.dram_tensor("bq", [CW], F32, kind="ExternalInput")
    bk = nc.dram_tensor("bk", [CW], F32, kind="ExternalInput")
    bv = nc.dram_tensor("bv", [CW], F32, kind="ExternalInput")
    out = nc.dram_tensor("out", [N, DIM], BF16, kind="ExternalOutput")

    with tile.TileContext(nc) as tc:
        with (
            tc.tile_pool(name="persist", bufs=1) as pp,
            tc.tile_pool(name="work", bufs=2) as wp,
            tc.tile_pool(name="ps_small", bufs=4, space="PSUM") as ps_s,
            tc.tile_pool(name="ps_big", bufs=2, space="PSUM") as ps_b,
        ):
            # ---- persistent SBUF tensors -------------------------------
            wq_sb = pp.tile([128, EC, CW], BF16, tag="wq")   # later: holds wo
            wk_sb = pp.tile([128, EC, CW], BF16, tag="wk")
            wv_sb = pp.tile([128, EC, CW], BF16, tag="wv")
            qT_sb = pp.tile([128, DC, N], BF16, tag="qT")
            kT_z = pp.tile([128, DC, 2, M], BF16, tag="kTz")
            v_sb = pp.tile([128, MC, HPC, 128], BF16, tag="v")
            # pT is an 8-slot ring (slot = mc % 8): PV subchunks consume the
            # exps inside the mc loop, so only a window of slots stays live
            pT_sb = [pp.tile([128, 8, 1024], BF16, tag="pT_e", name="pT_e"),
                     pp.tile([128, 8, 1024], BF16, tag="pT_o", name="pT_o")]
            st_sb = pp.tile([128, DC, N], BF16, tag="stacked")
            # flash-PV accumulators, one per (hp, ns) group of the live pair.
            # Two base-0 tiles per group (denominator rows / O^T rows): the
            # walrus verifier requires all SBUF APs of a DVE op to share the
            # same start partition, so partition-64-based views are out.
            oaccS = [pp.tile([64, 512], F32, tag=f"oaccS{g}", name=f"oaccS{g}")
                     for g in range(4)]
            oaccO = [pp.tile([64, 512], F32, tag=f"oaccO{g}", name=f"oaccO{g}")
                     for g in range(4)]
            bq_sb = pp.tile([128, DC], F32, tag="bq")
            bk_sb = pp.tile([128, DC], F32, tag="bk")
            bv1_sb = pp.tile([1, CW], F32, tag="bv1")
            bvb_sb = pp.tile([128, CW], F32, tag="bvb")

            # weights/biases go on the gpsimd DMA queue so activation-chunk
            # loads on the sync queue run in parallel with them.  The dc=0
            # slice of wk is sliced out first so the very first kt_group can
            # start as soon as ~300KB (not 2MB+) has landed.
            rwk = wk.ap().rearrange("(e p) c -> p e c", p=128)
            nc.gpsimd.dma_start(wk_sb[:, :, 0:128], rwk[:, :, 0:128])
            nc.gpsimd.dma_start(bk_sb[:], bk.ap().rearrange("(d p) -> p d", p=128))
            nc.gpsimd.dma_start(bv1_sb[:], bv.ap().rearrange("(a c) -> a c", a=1))
            nc.gpsimd.dma_start(wk_sb[:, :, 128:512], rwk[:, :, 128:512])
            rwq = wq.ap().rearrange("(e p) c -> p e c", p=128)
            nc.gpsimd.dma_start(wq_sb[:, :, 0:128], rwq[:, :, 0:128])
            nc.gpsimd.dma_start(bq_sb[:], bq.ap().rearrange("(d p) -> p d", p=128))
            nc.gpsimd.dma_start(wq_sb[:, :, 128:512], rwq[:, :, 128:512])
            nc.gpsimd.dma_start(wv_sb[:], wv.ap().rearrange("(e p) c -> p e c", p=128))
            # init work on the (otherwise idle) gpsimd engine: the vector
            # queue must stay free for the kt/qt bias-adds that gate QK
            nc.gpsimd.memset(kT_z[64:128, :, 0, :], 0.0)
            nc.gpsimd.memset(kT_z[0:64, :, 1, :], 0.0)
            nc.gpsimd.memset(v_sb[:, :, :, 0:HD], 1.0)
            # broadcast bv across partitions: [1,512] -> [128,512]
            nc.gpsimd.partition_broadcast(bvb_sb[:], bv1_sb[:])

            def load_chunk(src, nb):
                # split in two DMAs for finer dependency granularity
                chunk = wp.tile([128, EC, 512], BF16, tag="src_chunk")
                rsrc = src.ap().rearrange("(e p) n -> p e n", p=128)
                for half in range(2):
                    nc.sync.dma_start(
                        chunk[:, half * 4:(half + 1) * 4, :],
                        rsrc[:, half * 4:(half + 1) * 4,
                             nb * 512:(nb + 1) * 512],
                    )
                return chunk

            # (hp, ns) emission order for PV subchunks: ns=0 groups first so
            # the last pair can release its first out-projection rows early
            GROUPS = [(0, 0), (1, 0), (0, 1), (1, 1)]

            def attention_pair(dc, nb2, bg=(), tail_ops=None):
                # `bg` is a list of background emitters (projection /
                # out-projection psum groups) paced through the mc loop so
                # the PE always has independent work while ScalarE chews
                # through the exps.  PV is streamed *inside* the loop as
                # 4-mc subchunks (lagging 4 slots behind QK) that drain via
                # DVE into the SBUF accumulators oacc[g]; `tail_ops` is an
                # optional pair of emitter lists run after the ns=0 / ns=1
                # normalizations (tail out-projections of the last pair).
                bg = list(bg)
                emitted = 0

                def pv_sub(k, g):
                    hp, ns = GROUPS[g]
                    h = 2 * dc + hp
                    # lhsT = [ones*64 | v_h]: partitions 0-63 of the result
                    # all equal sum_m exp(S) (free in-matmul broadcast of
                    # the softmax denominator), partitions 64-127 are O^T.
                    po = ps_s.tile([128, 512], F32, tag="po")
                    for mc in range(4 * k, 4 * k + 4):
                        nc.tensor.matmul(
                            po[:],
                            v_sb[:, mc, h, :],
                            pT_sb[hp][:, mc % 8, ns * 512:(ns + 1) * 512],
                            start=(mc == 4 * k), stop=(mc == 4 * k + 3),
                        )
                    # PSUM reads must stay on DVE (gpsimd cannot touch PSUM)
                    if k == 0:
                        nc.vector.tensor_copy(out=oaccS[g][:], in_=po[0:64, :])
                        nc.vector.tensor_copy(out=oaccO[g][:], in_=po[64:128, :])
                    else:
                        nc.vector.tensor_tensor(
                            out=oaccS[g][:], in0=po[0:64, :], in1=oaccS[g][:],
                            op=mybir.AluOpType.add,
                        )
                        nc.vector.tensor_tensor(
                            out=oaccO[g][:], in0=po[64:128, :], in1=oaccO[g][:],
                            op=mybir.AluOpType.add,
                        )

                def norm(g):
                    hp, ns = GROUPS[g]
                    rbc = wp.tile([64, 512], F32, tag="rbc")
                    nc.vector.reciprocal_approx_fast(
                        out=rbc[:], in_=oaccS[g][:])
                    nsl = slice(nb2 * 1024 + ns * 512,
                                nb2 * 1024 + (ns + 1) * 512)
                    # the normalize multiply is SBUF-only, so it can ride on
                    # the otherwise idle gpsimd engine
                    if hp == 0:
                        nc.gpsimd.tensor_tensor(
                            out=st_sb[0:64, dc, nsl],
                            in0=oaccO[g][:], in1=rbc[:],
                            op=mybir.AluOpType.mult,
                        )
                    else:
                        tmp = wp.tile([64, 512], BF16, tag="otmp")
                        nc.gpsimd.tensor_tensor(
                            out=tmp[:], in0=oaccO[g][:],
                            in1=rbc[:],
                            op=mybir.AluOpType.mult,
                        )
                        nc.sync.dma_start(st_sb[64:128, dc, nsl], tmp[:])

                for mc in range(MC):
                    stp = [ps_b.tile([128, 1024], F32, tag="st", name=f"st{i}")
                           for i in range(2)]
                    for hf in range(2):
                        for hp in range(2):
                            nc.tensor.matmul(
                                stp[hp][:, hf * 512:(hf + 1) * 512],
                                kT_z[:, dc, hp, mc * 128:(mc + 1) * 128],
                                qT_sb[:, dc,
                                      nb2 * 1024 + hf * 512:
                                      nb2 * 1024 + (hf + 1) * 512],
                                start=True, stop=True,
                            )
                    for hp in range(2):
                        nc.scalar.activation(
                            pT_sb[hp][:, mc % 8, :], stp[hp][:],
                            mybir.ActivationFunctionType.Exp,
                            scale=SCALE,
                        )
                    if mc >= 4:
                        s = mc - 4
                        pv_sub(s // 4, s % 4)
                    while emitted < len(bg) * (mc + 1) // MC:
                        bg[emitted]()
                        emitted += 1
                # tail: last subchunk (mc 12-15), normalizations, tail ops
                pv_sub(3, 0)
                pv_sub(3, 1)
                norm(0)
                norm(1)
                if tail_ops:
                    for em in tail_ops[0]:
                        em()
                pv_sub(3, 2)
                pv_sub(3, 3)
                norm(2)
                norm(3)
                if tail_ops:
                    for em in tail_ops[1]:
                        em()

            def op_group(nck, jb, tail=False):
                # out-projection; wo lives in wq_sb (aliased after last qt use)
                acc = ps_s.tile([128, 512], F32, tag="po")
                for cc in range(DC):
                    nc.tensor.matmul(
                        acc[:],
                        st_sb[:, cc, nck * 128:(nck + 1) * 128],
                        wq_sb[:, cc * 2 + jb, :],
                        start=(cc == 0), stop=(cc == DC - 1),
                    )
                ot = wp.tile([128, 512], BF16, tag="out")
                # tail groups drain on gpsimd (vector is norm-congested at
                # the end) and the final DMAs alternate queues
                if tail:
                    nc.scalar.copy(out=ot[:], in_=acc[:])
                else:
                    nc.vector.tensor_copy(out=ot[:], in_=acc[:])
                dq = nc.gpsimd if (nck + jb) % 2 else nc.sync
                dq.dma_start(
                    out.ap()[nck * 128:(nck + 1) * 128,
                             jb * 512:(jb + 1) * 512],
                    ot[:],
                )

            def kt_group(dc, nb, chunk, eng=None):
                eng = eng or nc.vector
                acc = ps_s.tile([128, 512], F32, tag="po")
                for ec in range(EC):
                    nc.tensor.matmul(
                        acc[:],
                        wk_sb[:, ec, dc * 128:(dc + 1) * 128],
                        chunk[:, ec, :],
                        start=(ec == 0), stop=(ec == EC - 1),
                    )
                nsl = slice(nb * 512, (nb + 1) * 512)
                # write the two 64-row head halves into their padded slots
                eng.tensor_scalar_add(
                    kT_z[0:64, dc, 0, nsl], acc[0:64, :],
                    bk_sb[0:64, dc:dc + 1],
                )
                eng.tensor_scalar_add(
                    kT_z[64:128, dc, 1, nsl], acc[64:128, :],
                    bk_sb[64:128, dc:dc + 1],
                )

            def qt_group(dc, nb, chunk, eng=None):
                eng = eng or nc.vector
                acc = ps_s.tile([128, 512], F32, tag="po")
                for ec in range(EC):
                    nc.tensor.matmul(
                        acc[:],
                        wq_sb[:, ec, dc * 128:(dc + 1) * 128],
                        chunk[:, ec, :],
                        start=(ec == 0), stop=(ec == EC - 1),
                    )
                eng.tensor_scalar_add(
                    qT_sb[:, dc, nb * 512:(nb + 1) * 512],
                    acc[:], bq_sb[:, dc:dc + 1],
                )

            def v_group(mc, chunk, eng=None):
                eng = eng or nc.vector
                mi = mc % 4
                accv = ps_s.tile([128, 512], F32, tag="po")
                for ec in range(EC):
                    nc.tensor.matmul(
                        accv[:],
                        chunk[:, ec, mi * 128:(mi + 1) * 128],
                        wv_sb[:, ec, :],
                        start=(ec == 0), stop=(ec == EC - 1),
                    )
                eng.tensor_tensor(
                    out=v_sb[:, mc, :, HD:128],
                    in0=accv[:].rearrange("p (h d) -> p h d", h=HPC),
                    in1=bvb_sb[:].rearrange("p (h d) -> p h d", h=HPC),
                    op=mybir.AluOpType.add,
                )

            def proj_unit(fn, dc, nb, src):
                def emit():
                    fn(dc, nb, load_chunk(src, nb))
                return emit

            def wo_load_unit():
                # overwrite wq_sb with wo once the last qt_group has read it
                def emit():
                    nc.gpsimd.dma_start(
                        wq_sb[:].rearrange("p (e a) b -> p e a b", a=2),
                        wo.ap().rearrange("(e p) (a b) -> p e a b",
                                          p=128, b=512),
                    )
                return emit

            # ---- prefix: kT[0] + qT[0], then attention with all other
            # projection / output work paced through the mc loops as
            # background PE groups.
            # prefix drains go on vector: gpsimd is chewing through the init
            # memsets at this point and would delay the kT/qT writes
            for nb in range(NB):
                kt_group(0, nb, load_chunk(cT, nb), eng=nc.vector)
            for nb in range(NB):
                qt_group(0, nb, load_chunk(xT, nb), eng=nc.vector)

            vcache = {}
            def v_unit2(mc):
                def emit():
                    nb = mc // 4
                    if vcache.get("nb") != nb:
                        vcache["nb"] = nb
                        vcache["c"] = load_chunk(cT, nb)
                    v_group(mc, vcache["c"])
                return emit

            bg_plan = {
                (0, 0): [v_unit2(mc) for mc in range(MC)],
                (0, 1): [proj_unit(kt_group, 1, nb, cT) for nb in range(NB)]
                      + [proj_unit(qt_group, 1, nb, xT) for nb in range(NB)],
                (1, 0): [proj_unit(kt_group, 2, nb, cT) for nb in range(NB)],
                (1, 1): [proj_unit(qt_group, 2, nb, xT) for nb in range(NB)],
                (2, 0): [proj_unit(kt_group, 3, nb, cT) for nb in range(NB)],
                (2, 1): [proj_unit(qt_group, 3, nb, xT) for nb in range(NB)]
                      + [wo_load_unit()],
                (3, 1): [(lambda nck=nck, jb=jb: op_group(nck, jb))
                         for nck in range(0, 8) for jb in range(2)],
            }
            # last pair: pull half of the remaining out-projection into the
            # pair tail (nck 8-11 only needs the ns=0 st rows, 12-15 ns=1)
            tail_ops_last = (
                [(lambda nck=nck, jb=jb: op_group(nck, jb, tail=True))
                 for nck in range(8, 12) for jb in range(2)],
                [(lambda nck=nck, jb=jb: op_group(nck, jb, tail=True))
                 for nck in range(12, 16) for jb in range(2)],
            )
            for dc in range(DC):
                for nb2 in range(2):
                    attention_pair(
                        dc, nb2, bg_plan.get((dc, nb2), ()),
                        tail_ops=tail_ops_last if (dc, nb2) == (3, 1) else None,
                    )
    nc.compile()
    return nc


_NC_CACHE = None


def _get_nc():
    global _NC_CACHE
    if _NC_CACHE is None:
        _NC_CACHE = _build_nc()
    return _NC_CACHE


def make_in_maps(x, context, Wq, bq, Wk, bk, Wv, bv, Wo, bo):
    """Host-side sharding: per-core transposed bf16 operand prep."""
    in_maps = []
    for c in range(N_CORES):
        b, hg = c // 2, c % 2
        cs = slice(hg * CW, hg * CW + CW)
        in_maps.append({
            "xT": np.ascontiguousarray(x[b].T).astype(nbf),
            "cT": np.ascontiguousarray(context[b].T).astype(nbf),
            "wq": np.ascontiguousarray(Wq[cs].T).astype(nbf),
            "wk": np.ascontiguousarray(Wk[cs].T).astype(nbf),
            "wv": np.ascontiguousarray(Wv[cs].T).astype(nbf),
            "wo": np.ascontiguousarray(Wo[:, cs].T).astype(nbf),
            "bq": np.ascontiguousarray(bq[cs]).astype(np.float32),
            "bk": np.ascontiguousarray(bk[cs]).astype(np.float32),
            "bv": np.ascontiguousarray(bv[cs]).astype(np.float32),
        })
    return in_maps


def gather(results, bo):
    """Host-side unshard: sum the two head-group partials per batch, add bo."""
    out = np.empty((B, N, DIM), np.float32)
    for b in range(B):
        out[b] = (results[2 * b]["out"].astype(np.float32)
                  + results[2 * b + 1]["out"].astype(np.float32))
    out += np.asarray(bo, np.float32)[None, None, :]
    return out


def kernel(x, context, Wq, bq, Wk, bk, Wv, bv, Wo, bo):
    nc = _get_nc()
    in_maps = make_in_maps(x, context, Wq, bq, Wk, bk, Wv, bv, Wo, bo)
    res = run_bass_kernel_spmd(nc, in_maps, list(range(N_CORES)))
    return gather(res.results, bo)


# revision 32
# speedup vs baseline: 1.0217x; 1.0217x over previous
"""Trainium2 Bass kernel for nn_CrossAttention (B=4, N=M=2048, DIM=1024, H=16, D=64).

Sharding: batch x head-group over 8 cores. Core c handles batch b = c//2 and
heads hgrp = c%2 (8 heads, a contiguous 512-wide slice of the hidden dim).
Each core computes q/k/v projections for its heads, flash-style attention in
S^T layout (keys on partitions), and a *partial* output projection over its
512 hidden dims. The host sums the two partials per batch and adds the output
bias (the only cross-core reduction).

Device layouts (chosen so every matmul has its contraction dim on partitions):
  xT/cT   [E=1024, N=2048]   (host-transposed, bf16)
  qT      [512, 2048]        d-on-partitions, produced by lhsT=W^T, rhs=xT
  kT_z    [128, DC, 2, M]    zero-padded per head-of-pair: slot hp holds that
                             head's 64 d-rows, the other 64 rows are 0.  Every
                             QK matmul then uses a full 128-row stationary so
                             ALL matmuls share one (128,128) PE tile config —
                             mixed 64/128 configs cost ~90ns per transition.
  v_aug   [m, head, 65]      keys-on-partitions; col 64 == 1.0 so the PV
                             matmul also yields the softmax denominator
  S^T     [m=128, n=1024]    PSUM; exp on ScalarE (scale=1/8 fused)
  O^T     [65, 512] PSUM     partition 64 = sum_m exp(S); normalization via a
                             K=1 broadcast matmul + reciprocal + multiply
"""

import numpy as np
import ml_dtypes

import concourse.bass as bass
import concourse.mybir as mybir
import concourse.tile as tile
from concourse import bacc
from concourse.bass_utils import run_bass_kernel_spmd

N_CORES = 8
B, N, M, DIM = 4, 2048, 2048, 1024
HEADS, HD = 16, 64            # total heads, head dim
HPC = 8                       # heads per core
CW = HPC * HD                 # per-core hidden width = 512
EC = DIM // 128               # 8 contraction chunks of 128
DC = CW // 128                # 4 chunks of the per-core q/k dims
MC = M // 128                 # 16 key chunks
NB = N // 512                 # 4 query 512-blocks
SCALE = HD ** -0.5            # 0.125

F32 = mybir.dt.float32
BF16 = mybir.dt.bfloat16
nbf = ml_dtypes.bfloat16


def _build_nc():
    nc = bacc.Bacc("TRN2", target_bir_lowering=False, debug=False,
                   num_devices=N_CORES)

    xT = nc.dram_tensor("xT", [DIM, N], BF16, kind="ExternalInput")
    cT = nc.dram_tensor("cT", [DIM, M], BF16, kind="ExternalInput")
    # weights come in pre-permuted to the exact SBUF layout (partition-major)
    # so every weight DMA moves whole contiguous per-partition rows
    wq = nc.dram_tensor("wq", [128, DC, EC, 128], BF16, kind="ExternalInput")
    wk = nc.dram_tensor("wk", [128, DC, EC, 128], BF16, kind="ExternalInput")
    wv = nc.dram_tensor("wv", [128, EC, CW], BF16, kind="ExternalInput")
    wo = nc.dram_tensor("wo", [128, DC, EC, 128], BF16, kind="ExternalInput")
    bq = nc.dram_tensor("bq", [CW], F32, kind="ExternalInput")
    bk = nc.dram_tensor("bk", [CW], F32, kind="ExternalInput")
    bv = nc.dram_tensor("bv", [CW], F32, kind="ExternalInput")
    out = nc.dram_tensor("out", [N, DIM], BF16, kind="ExternalOutput")

    with tile.TileContext(nc) as tc:
        with (
            tc.tile_pool(name="persist", bufs=1) as pp,
            tc.tile_pool(name="work", bufs=2) as wp,
            tc.tile_pool(name="ps_small", bufs=4, space="PSUM") as ps_s,
            tc.tile_pool(name="ps_big", bufs=2, space="PSUM") as ps_b,
        ):
            # ---- persistent SBUF tensors -------------------------------
            wq_sb = pp.tile([128, DC, EC, 128], BF16, tag="wq")  # later: wo
            wk_sb = pp.tile([128, DC, EC, 128], BF16, tag="wk")
            wv_sb = pp.tile([128, EC, CW], BF16, tag="wv")
            qT_sb = pp.tile([128, DC, N], BF16, tag="qT")
            kT_z = pp.tile([128, DC, 2, M], BF16, tag="kTz")
            v_sb = pp.tile([128, MC, HPC, 128], BF16, tag="v")
            # pT is an 8-slot ring (slot = mc % 8): PV subchunks consume the
            # exps inside the mc loop, so only a window of slots stays live
            pT_sb = [pp.tile([128, 8, 1024], BF16, tag="pT_e", name="pT_e"),
                     pp.tile([128, 8, 1024], BF16, tag="pT_o", name="pT_o")]
            st_sb = pp.tile([128, DC, N], BF16, tag="stacked")
            # flash-PV accumulators, one per (hp, ns) group of the live pair.
            # Two base-0 tiles per group (denominator rows / O^T rows): the
            # walrus verifier requires all SBUF APs of a DVE op to share the
            # same start partition, so partition-64-based views are out.
            oaccS = [pp.tile([64, 512], F32, tag=f"oaccS{g}", name=f"oaccS{g}")
                     for g in range(4)]
            oaccO = [pp.tile([64, 512], F32, tag=f"oaccO{g}", name=f"oaccO{g}")
                     for g in range(4)]
            bq_sb = pp.tile([128, DC], F32, tag="bq")
            bk_sb = pp.tile([128, DC], F32, tag="bk")
            bv1_sb = pp.tile([1, CW], F32, tag="bv1")
            bvb_sb = pp.tile([128, CW], F32, tag="bvb")

            # weights/biases go on the gpsimd DMA queue so activation-chunk
            # loads on the sync queue run in parallel with them.  The dc=0
            # slice of wk is sliced out first so the very first kt_group can
            # start as soon as ~300KB (not 2MB+) has landed.
            nc.gpsimd.dma_start(wk_sb[:, 0], wk.ap()[:, 0])
            nc.gpsimd.dma_start(bk_sb[:], bk.ap().rearrange("(d p) -> p d", p=128))
            nc.gpsimd.dma_start(wq_sb[:, 0], wq.ap()[:, 0])
            nc.gpsimd.dma_start(bq_sb[:], bq.ap().rearrange("(d p) -> p d", p=128))
            nc.gpsimd.dma_start(bv1_sb[:], bv.ap().rearrange("(a c) -> a c", a=1))
            nc.gpsimd.dma_start(wk_sb[:, 1:DC], wk.ap()[:, 1:DC])
            nc.gpsimd.dma_start(wq_sb[:, 1:DC], wq.ap()[:, 1:DC])
            nc.gpsimd.dma_start(wv_sb[:], wv.ap())
            # init work on the (otherwise idle) gpsimd engine: the vector
            # queue must stay free for the kt/qt bias-adds that gate QK
            nc.gpsimd.memset(kT_z[64:128, :, 0, :], 0.0)
            nc.gpsimd.memset(kT_z[0:64, :, 1, :], 0.0)
            nc.gpsimd.memset(v_sb[:, :, :, 0:HD], 1.0)
            # broadcast bv across partitions: [1,512] -> [128,512]
            nc.gpsimd.partition_broadcast(bvb_sb[:], bv1_sb[:])

            def load_chunk(src, nb, q=None):
                # split in two DMAs for finer dependency granularity
                chunk = wp.tile([128, EC, 512], BF16, tag="src_chunk")
                rsrc = src.ap().rearrange("(e p) n -> p e n", p=128)
                for half in range(2):
                    (q or nc.sync).dma_start(
                        chunk[:, half * 4:(half + 1) * 4, :],
                        rsrc[:, half * 4:(half + 1) * 4,
                             nb * 512:(nb + 1) * 512],
                    )
                return chunk

            # (hp, ns) emission order for PV subchunks: ns=0 groups first so
            # the last pair can release its first out-projection rows early
            GROUPS = [(0, 0), (1, 0), (0, 1), (1, 1)]

            def attention_pair(dc, nb2, bg=(), tail_ops=None):
                # `bg` is a list of background emitters (projection /
                # out-projection psum groups) paced through the mc loop so
                # the PE always has independent work while ScalarE chews
                # through the exps.  PV is streamed *inside* the loop as
                # 4-mc subchunks (lagging 4 slots behind QK) that drain via
                # DVE into the SBUF accumulators oacc[g]; `tail_ops` is an
                # optional pair of emitter lists run after the ns=0 / ns=1
                # normalizations (tail out-projections of the last pair).
                bg = list(bg)
                emitted = 0

                def pv_sub(k, g):
                    hp, ns = GROUPS[g]
                    h = 2 * dc + hp
                    # lhsT = [ones*64 | v_h]: partitions 0-63 of the result
                    # all equal sum_m exp(S) (free in-matmul broadcast of
                    # the softmax denominator), partitions 64-127 are O^T.
                    po = ps_s.tile([128, 512], F32, tag="po")
                    for mc in range(4 * k, 4 * k + 4):
                        nc.tensor.matmul(
                            po[:],
                            v_sb[:, mc, h, :],
                            pT_sb[hp][:, mc % 8, ns * 512:(ns + 1) * 512],
                            start=(mc == 4 * k), stop=(mc == 4 * k + 3),
                        )
                    # PSUM reads must stay on DVE (gpsimd cannot touch PSUM)
                    if k == 0:
                        nc.vector.tensor_copy(out=oaccS[g][:], in_=po[0:64, :])
                        nc.vector.tensor_copy(out=oaccO[g][:], in_=po[64:128, :])
                    else:
                        nc.vector.tensor_tensor(
                            out=oaccS[g][:], in0=po[0:64, :], in1=oaccS[g][:],
                            op=mybir.AluOpType.add,
                        )
                        nc.vector.tensor_tensor(
                            out=oaccO[g][:], in0=po[64:128, :], in1=oaccO[g][:],
                            op=mybir.AluOpType.add,
                        )

                def norm(g):
                    hp, ns = GROUPS[g]
                    rbc = wp.tile([64, 512], F32, tag="rbc")
                    nc.vector.reciprocal_approx_fast(
                        out=rbc[:], in_=oaccS[g][:])
                    nsl = slice(nb2 * 1024 + ns * 512,
                                nb2 * 1024 + (ns + 1) * 512)
                    if hp == 0:
                        nc.vector.tensor_tensor(
                            out=st_sb[0:64, dc, nsl],
                            in0=oaccO[g][:], in1=rbc[:],
                            op=mybir.AluOpType.mult,
                        )
                    else:
                        tmp = wp.tile([64, 512], BF16, tag="otmp")
                        nc.vector.tensor_tensor(
                            out=tmp[:], in0=oaccO[g][:],
                            in1=rbc[:],
                            op=mybir.AluOpType.mult,
                        )
                        nc.sync.dma_start(st_sb[64:128, dc, nsl], tmp[:])

                for mc in range(MC):
                    stp = [ps_b.tile([128, 1024], F32, tag="st", name=f"st{i}")
                           for i in range(2)]
                    # hp-outer so each head's exp can start as soon as that
                    # head's two matmuls land (ScalarE gets a head start)
                    for hp in range(2):
                        for hf in range(2):
                            nc.tensor.matmul(
                                stp[hp][:, hf * 512:(hf + 1) * 512],
                                kT_z[:, dc, hp, mc * 128:(mc + 1) * 128],
                                qT_sb[:, dc,
                                      nb2 * 1024 + hf * 512:
                                      nb2 * 1024 + (hf + 1) * 512],
                                start=True, stop=True,
                            )
                        nc.scalar.activation(
                            pT_sb[hp][:, mc % 8, :], stp[hp][:],
                            mybir.ActivationFunctionType.Exp,
                            scale=SCALE,
                        )
                    if mc >= 4:
                        s = mc - 4
                        pv_sub(s // 4, s % 4)
                    while emitted < len(bg) * (mc + 1) // MC:
                        bg[emitted]()
                        emitted += 1
                # tail: last subchunk (mc 12-15), normalizations, tail ops
                pv_sub(3, 0)
                pv_sub(3, 1)
                norm(0)
                norm(1)
                if tail_ops:
                    for em in tail_ops[0]:
                        em()
                pv_sub(3, 2)
                pv_sub(3, 3)
                norm(2)
                norm(3)
                if tail_ops:
                    for em in tail_ops[1]:
                        em()

            def op_group(nck, jb, tail=False):
                # out-projection; wo lives in wq_sb (aliased after last qt
                # use).  Tail groups alternate between the small-psum ring
                # and the (by then idle) big-psum pool for a deeper acc ring.
                if tail and (nck + jb) % 2:
                    acc = ps_b.tile([128, 1024], F32, tag="st", name="st0")
                    acc_ap = acc[:, 0:512]
                else:
                    acc = ps_s.tile([128, 512], F32, tag="po", name="po")
                    acc_ap = acc[:]
                for cc in range(DC):
                    nc.tensor.matmul(
                        acc_ap,
                        st_sb[:, cc, nck * 128:(nck + 1) * 128],
                        wq_sb[:, cc, 4 * jb:4 * jb + 4, :],
                        start=(cc == 0), stop=(cc == DC - 1),
                    )
                ot = wp.tile([128, 512], BF16, tag="out")
                # tail copies go to the (idle-by-then) scalar engine and the
                # final DMAs alternate queues
                if tail:
                    nc.scalar.copy(out=ot[:], in_=acc_ap)
                else:
                    nc.vector.tensor_copy(out=ot[:], in_=acc_ap)
                dq = nc.gpsimd if (nck + jb) % 2 else nc.sync
                dq.dma_start(
                    out.ap()[nck * 128:(nck + 1) * 128,
                             jb * 512:(jb + 1) * 512],
                    ot[:],
                )

            def kt_group(dc, nb, chunk, eng=None):
                eng = eng or nc.vector
                acc = ps_s.tile([128, 512], F32, tag="po")
                for ec in range(EC):
                    nc.tensor.matmul(
                        acc[:],
                        wk_sb[:, dc, ec, :],
                        chunk[:, ec, :],
                        start=(ec == 0), stop=(ec == EC - 1),
                    )
                nsl = slice(nb * 512, (nb + 1) * 512)
                # write the two 64-row head halves into their padded slots
                eng.tensor_scalar_add(
                    kT_z[0:64, dc, 0, nsl], acc[0:64, :],
                    bk_sb[0:64, dc:dc + 1],
                )
                eng.tensor_scalar_add(
                    kT_z[64:128, dc, 1, nsl], acc[64:128, :],
                    bk_sb[64:128, dc:dc + 1],
                )

            def qt_group(dc, nb, chunk, eng=None):
                eng = eng or nc.vector
                acc = ps_s.tile([128, 512], F32, tag="po")
                for ec in range(EC):
                    nc.tensor.matmul(
                        acc[:],
                        wq_sb[:, dc, ec, :],
                        chunk[:, ec, :],
                        start=(ec == 0), stop=(ec == EC - 1),
                    )
                eng.tensor_scalar_add(
                    qT_sb[:, dc, nb * 512:(nb + 1) * 512],
                    acc[:], bq_sb[:, dc:dc + 1],
                )

            def v_group(mc, chunk, eng=None):
                eng = eng or nc.vector
                mi = mc % 4
                accv = ps_s.tile([128, 512], F32, tag="po")
                for ec in range(EC):
                    nc.tensor.matmul(
                        accv[:],
                        chunk[:, ec, mi * 128:(mi + 1) * 128],
                        wv_sb[:, ec, :],
                        start=(ec == 0), stop=(ec == EC - 1),
                    )
                eng.tensor_tensor(
                    out=v_sb[:, mc, :, HD:128],
                    in0=accv[:].rearrange("p (h d) -> p h d", h=HPC),
                    in1=bvb_sb[:].rearrange("p (h d) -> p h d", h=HPC),
                    op=mybir.AluOpType.add,
                )

            def proj_unit(fn, dc, nb, src):
                def emit():
                    fn(dc, nb, load_chunk(src, nb))
                return emit

            def wo_load_unit():
                # overwrite wq_sb with wo once the last qt_group has read it
                def emit():
                    nc.gpsimd.dma_start(wq_sb[:], wo.ap())
                return emit

            # ---- prefix: kT[0] + qT[0], then attention with all other
            # projection / output work paced through the mc loops as
            # background PE groups.
            # prefix drains go on vector: gpsimd is chewing through the init
            # memsets at this point and would delay the kT/qT writes.  The
            # prefix is DMA-bound, so chunk loads alternate between the sync
            # and (idle until the first exp) scalar DMA queues, and only the
            # slices pair (0,0) actually reads are produced here.
            for nb in range(NB):
                kt_group(0, nb, load_chunk(cT, nb, q=nc.scalar if nb % 2
                                           else nc.sync), eng=nc.vector)
            for nb in range(2):
                qt_group(0, nb, load_chunk(xT, nb, q=nc.scalar if nb % 2
                                           else nc.sync), eng=nc.vector)

            vcache = {}
            def v_unit2(mc):
                def emit():
                    nb = mc // 4
                    if vcache.get("nb") != nb:
                        vcache["nb"] = nb
                        vcache["c"] = load_chunk(cT, nb)
                    v_group(mc, vcache["c"])
                return emit

            bg_plan = {
                (0, 0): [v_unit2(mc) for mc in range(MC)]
                      + [proj_unit(qt_group, 0, 2, xT),
                         proj_unit(qt_group, 0, 3, xT)],
                (0, 1): [proj_unit(kt_group, 1, nb, cT) for nb in range(NB)]
                      + [proj_unit(qt_group, 1, nb, xT) for nb in range(NB)],
                (1, 0): [proj_unit(kt_group, 2, nb, cT) for nb in range(NB)],
                (1, 1): [proj_unit(qt_group, 2, nb, xT) for nb in range(NB)],
                (2, 0): [proj_unit(kt_group, 3, nb, cT) for nb in range(NB)],
                (2, 1): [proj_unit(qt_group, 3, nb, xT) for nb in range(NB)]
                      + [wo_load_unit()],
                (3, 1): [(lambda nck=nck, jb=jb: op_group(nck, jb))
                         for nck in range(0, 8) for jb in range(2)],
            }
            # last pair: pull half of the remaining out-projection into the
            # pair tail (nck 8-11 only needs the ns=0 st rows, 12-15 ns=1)
            tail_ops_last = (
                [(lambda nck=nck, jb=jb: op_group(nck, jb, tail=True))
                 for nck in range(8, 12) for jb in range(2)],
                [(lambda nck=nck, jb=jb: op_group(nck, jb, tail=True))
                 for nck in range(12, 16) for jb in range(2)],
            )
            for dc in range(DC):
                for nb2 in range(2):
                    attention_pair(
                        dc, nb2, bg_plan.get((dc, nb2), ()),
                        tail_ops=tail_ops_last if (dc, nb2) == (3, 1) else None,
                    )
    nc.compile()
    return nc


_NC_CACHE = None


def _get_nc():
    global _NC_CACHE
    if _NC_CACHE is None:
        _NC_CACHE = _build_nc()
    return _NC_CACHE


def _wqk_layout(W):
    """[512, 1024] -> [128, DC, EC, 128]: element [p,dc,ec,c] = W[dc*128+c,
    ec*128+p] (partition from the contraction axis; q/k proj stationary)."""
    return np.ascontiguousarray(
        np.asarray(W).reshape(DC, 128, EC, 128).transpose(3, 0, 2, 1)
    ).astype(nbf)


def _wo_layout(W):
    """[512, 1024] -> [128, DC, EC, 128]: element [p,cc,ee,c] = W[cc*128+p,
    ee*128+c] (partition from the CW axis; out-proj moving operand)."""
    return np.ascontiguousarray(
        np.asarray(W).reshape(DC, 128, EC, 128).transpose(1, 0, 2, 3)
    ).astype(nbf)


def make_in_maps(x, context, Wq, bq, Wk, bk, Wv, bv, Wo, bo):
    """Host-side sharding: per-core transposed bf16 operand prep."""
    in_maps = []
    for c in range(N_CORES):
        b, hg = c // 2, c % 2
        cs = slice(hg * CW, hg * CW + CW)
        wv_host = np.ascontiguousarray(
            np.asarray(Wv[cs]).reshape(CW, EC, 128).transpose(2, 1, 0)
        ).astype(nbf)
        in_maps.append({
            "xT": np.ascontiguousarray(x[b].T).astype(nbf),
            "cT": np.ascontiguousarray(context[b].T).astype(nbf),
            "wq": _wqk_layout(Wq[cs]),
            "wk": _wqk_layout(Wk[cs]),
            "wv": wv_host,
            "wo": _wo_layout(np.asarray(Wo)[:, cs].T),
            "bq": np.ascontiguousarray(bq[cs]).astype(np.float32),
            "bk": np.ascontiguousarray(bk[cs]).astype(np.float32),
            "bv": np.ascontiguousarray(bv[cs]).astype(np.float32),
        })
    return in_maps


def gather(results, bo):
    """Host-side unshard: sum the two head-group partials per batch, add bo."""
    out = np.empty((B, N, DIM), np.float32)
    for b in range(B):
        out[b] = (results[2 * b]["out"].astype(np.float32)
                  + results[2 * b + 1]["out"].astype(np.float32))
    out += np.asarray(bo, np.float32)[None, None, :]
    return out


def kernel(x, context, Wq, bq, Wk, bk, Wv, bv, Wo, bo):
    nc = _get_nc()
    in_maps = make_in_maps(x, context, Wq, bq, Wk, bk, Wv, bv, Wo, bo)
    res = run_bass_kernel_spmd(nc, in_maps, list(range(N_CORES)))
    return gather(res.results, bo)


# revision 35
# speedup vs baseline: 1.0403x; 1.0182x over previous
"""Trainium2 Bass kernel for nn_CrossAttention (B=4, N=M=2048, DIM=1024, H=16, D=64).

Sharding: batch x head-group over 8 cores. Core c handles batch b = c//2 and
heads hgrp = c%2 (8 heads, a contiguous 512-wide slice of the hidden dim).
Each core computes q/k/v projections for its heads, flash-style attention in
S^T layout (keys on partitions), and a *partial* output projection over its
512 hidden dims. The host sums the two partials per batch and adds the output
bias (the only cross-core reduction).

Device layouts (chosen so every matmul has its contraction dim on partitions):
  xT/cT   [E=1024, N=2048]   (host-transposed, bf16)
  qT      [512, 2048]        d-on-partitions, produced by lhsT=W^T, rhs=xT
  kT_z    [128, DC, 2, M]    zero-padded per head-of-pair: slot hp holds that
                             head's 64 d-rows, the other 64 rows are 0.  Every
                             QK matmul then uses a full 128-row stationary so
                             ALL matmuls share one (128,128) PE tile config —
                             mixed 64/128 configs cost ~90ns per transition.
  v_aug   [m, head, 65]      keys-on-partitions; col 64 == 1.0 so the PV
                             matmul also yields the softmax denominator
  S^T     [m=128, n=1024]    PSUM; exp on ScalarE (scale=1/8 fused)
  O^T     [65, 512] PSUM     partition 64 = sum_m exp(S); normalization via a
                             K=1 broadcast matmul + reciprocal + multiply
"""

import numpy as np
import ml_dtypes

import concourse.bass as bass
import concourse.mybir as mybir
import concourse.tile as tile
from concourse import bacc
from concourse.bass_utils import run_bass_kernel_spmd

N_CORES = 8
B, N, M, DIM = 4, 2048, 2048, 1024
HEADS, HD = 16, 64            # total heads, head dim
HPC = 8                       # heads per core
CW = HPC * HD                 # per-core hidden width = 512
EC = DIM // 128               # 8 contraction chunks of 128
DC = CW // 128                # 4 chunks of the per-core q/k dims
MC = M // 128                 # 16 key chunks
NB = N // 512                 # 4 query 512-blocks
SCALE = HD ** -0.5            # 0.125

F32 = mybir.dt.float32
BF16 = mybir.dt.bfloat16
nbf = ml_dtypes.bfloat16


def _build_nc():
    nc = bacc.Bacc("TRN2", target_bir_lowering=False, debug=False,
                   num_devices=N_CORES)

    xT = nc.dram_tensor("xT", [DIM, N], BF16, kind="ExternalInput")
    cT = nc.dram_tensor("cT", [DIM, M], BF16, kind="ExternalInput")
    # weights come in pre-permuted to the exact SBUF layout (partition-major)
    # so every weight DMA moves whole contiguous per-partition rows
    wq = nc.dram_tensor("wq", [128, DC, EC, 128], BF16, kind="ExternalInput")
    wk = nc.dram_tensor("wk", [128, DC, EC, 128], BF16, kind="ExternalInput")
    wv = nc.dram_tensor("wv", [128, EC, CW], BF16, kind="ExternalInput")
    wo = nc.dram_tensor("wo", [128, DC, EC, 128], BF16, kind="ExternalInput")
    bq = nc.dram_tensor("bq", [CW], F32, kind="ExternalInput")
    bk = nc.dram_tensor("bk", [CW], F32, kind="ExternalInput")
    bv = nc.dram_tensor("bv", [CW], F32, kind="ExternalInput")
    out = nc.dram_tensor("out", [N, DIM], BF16, kind="ExternalOutput")

    with tile.TileContext(nc) as tc:
        with (
            tc.tile_pool(name="persist", bufs=1) as pp,
            tc.tile_pool(name="work", bufs=2) as wp,
            tc.tile_pool(name="ps_small", bufs=4, space="PSUM") as ps_s,
            tc.tile_pool(name="ps_big", bufs=2, space="PSUM") as ps_b,
        ):
            # ---- persistent SBUF tensors -------------------------------
            wq_sb = pp.tile([128, DC, EC, 128], BF16, tag="wq")  # later: wo
            wk_sb = pp.tile([128, DC, EC, 128], BF16, tag="wk")
            wv_sb = pp.tile([128, EC, CW], BF16, tag="wv")
            qT_sb = pp.tile([128, DC, N], BF16, tag="qT")
            kT_z = pp.tile([128, DC, 2, M], BF16, tag="kTz")
            v_sb = pp.tile([128, MC, HPC, 128], BF16, tag="v")
            # pT is an 8-slot ring (slot = mc % 8): PV subchunks consume the
            # exps inside the mc loop, so only a window of slots stays live
            pT_sb = [pp.tile([128, 8, 1024], BF16, tag="pT_e", name="pT_e"),
                     pp.tile([128, 8, 1024], BF16, tag="pT_o", name="pT_o")]
            st_sb = pp.tile([128, DC, N], BF16, tag="stacked")
            # flash-PV accumulators, one per (hp, ns) group of the live pair.
            # Two base-0 tiles per group (denominator rows / O^T rows): the
            # walrus verifier requires all SBUF APs of a DVE op to share the
            # same start partition, so partition-64-based views are out.
            oaccS = [pp.tile([64, 512], F32, tag=f"oaccS{g}",
                             name=f"oaccS{g}")[:] for g in range(4)]
            oaccO = [pp.tile([64, 512], F32, tag=f"oaccO{g}",
                             name=f"oaccO{g}")[:] for g in range(4)]
            bq_sb = pp.tile([128, DC], F32, tag="bq")
            bk_sb = pp.tile([128, DC], F32, tag="bk")
            bv1_sb = pp.tile([1, CW], F32, tag="bv1")
            bvb_sb = pp.tile([128, CW], F32, tag="bvb")

            # weights/biases go on the gpsimd DMA queue so activation-chunk
            # loads on the sync queue run in parallel with them.  The dc=0
            # slice of wk is sliced out first so the very first kt_group can
            # start as soon as ~300KB (not 2MB+) has landed.
            nc.gpsimd.dma_start(wk_sb[:, 0], wk.ap()[:, 0])
            nc.gpsimd.dma_start(bk_sb[:], bk.ap().rearrange("(d p) -> p d", p=128))
            nc.gpsimd.dma_start(wq_sb[:, 0], wq.ap()[:, 0])
            nc.gpsimd.dma_start(bq_sb[:], bq.ap().rearrange("(d p) -> p d", p=128))
            nc.gpsimd.dma_start(bv1_sb[:], bv.ap().rearrange("(a c) -> a c", a=1))
            nc.gpsimd.dma_start(wv_sb[:], wv.ap())
            nc.gpsimd.dma_start(wk_sb[:, 1:DC], wk.ap()[:, 1:DC])
            nc.gpsimd.dma_start(wq_sb[:, 1:DC], wq.ap()[:, 1:DC])
            # init work on the (otherwise idle) gpsimd engine: the vector
            # queue must stay free for the kt/qt bias-adds that gate QK
            nc.gpsimd.memset(kT_z[64:128, :, 0, :], 0.0)
            nc.gpsimd.memset(kT_z[0:64, :, 1, :], 0.0)
            nc.gpsimd.memset(v_sb[:, :, :, 0:HD], 1.0)
            # broadcast bv across partitions: [1,512] -> [128,512]
            nc.gpsimd.partition_broadcast(bvb_sb[:], bv1_sb[:])

            def load_chunk(src, nb, q=None):
                # split in two DMAs for finer dependency granularity
                chunk = wp.tile([128, EC, 512], BF16, tag="src_chunk")
                rsrc = src.ap().rearrange("(e p) n -> p e n", p=128)
                for half in range(2):
                    (q or nc.sync).dma_start(
                        chunk[:, half * 4:(half + 1) * 4, :],
                        rsrc[:, half * 4:(half + 1) * 4,
                             nb * 512:(nb + 1) * 512],
                    )
                return chunk

            # (hp, ns) emission order for PV subchunks: ns=0 groups first so
            # the last pair can release its first out-projection rows early
            GROUPS = [(0, 0), (1, 0), (0, 1), (1, 1)]

            def attention_pair(dc, nb2, bg=(), tail_ops=None):
                # `bg` is a list of background emitters (projection /
                # out-projection psum groups) paced through the mc loop so
                # the PE always has independent work while ScalarE chews
                # through the exps.  PV is streamed *inside* the loop as
                # 4-mc subchunks (lagging 4 slots behind QK) that drain via
                # DVE into the SBUF accumulators oacc[g]; `tail_ops` is an
                # optional pair of emitter lists run after the ns=0 / ns=1
                # normalizations (tail out-projections of the last pair).
                bg = list(bg)
                emitted = 0

                def pv_sub(k, g):
                    hp, ns = GROUPS[g]
                    h = 2 * dc + hp
                    # lhsT = [ones*64 | v_h]: partitions 0-63 of the result
                    # all equal sum_m exp(S) (free in-matmul broadcast of
                    # the softmax denominator), partitions 64-127 are O^T.
                    po = ps_s.tile([128, 512], F32, tag="po")
                    for mc in range(4 * k, 4 * k + 4):
                        nc.tensor.matmul(
                            po[:],
                            v_sb[:, mc, h, :],
                            pT_sb[hp][:, mc % 8, ns * 512:(ns + 1) * 512],
                            start=(mc == 4 * k), stop=(mc == 4 * k + 3),
                        )
                    # PSUM reads must stay on DVE (gpsimd cannot touch PSUM)
                    if k == 0:
                        nc.vector.tensor_copy(out=oaccS[g], in_=po[0:64, :])
                        nc.vector.tensor_copy(out=oaccO[g], in_=po[64:128, :])
                    else:
                        nc.vector.tensor_tensor(
                            out=oaccS[g], in0=po[0:64, :], in1=oaccS[g],
                            op=mybir.AluOpType.add,
                        )
                        nc.vector.tensor_tensor(
                            out=oaccO[g], in0=po[64:128, :], in1=oaccO[g],
                            op=mybir.AluOpType.add,
                        )

                def norm(g):
                    hp, ns = GROUPS[g]
                    rbct = wp.tile([64, 512], F32, tag="rbc")
                    rbc = rbct[:]
                    nc.vector.reciprocal_approx_fast(out=rbc, in_=oaccS[g])
                    nsl = slice(nb2 * 1024 + ns * 512,
                                nb2 * 1024 + (ns + 1) * 512)
                    if hp == 0:
                        nc.vector.tensor_tensor(
                            out=st_sb[0:64, dc, nsl],
                            in0=oaccO[g], in1=rbc,
                            op=mybir.AluOpType.mult,
                        )
                    else:
                        tmp = wp.tile([64, 512], BF16, tag="otmp")
                        nc.vector.tensor_tensor(
                            out=tmp[:], in0=oaccO[g], in1=rbc,
                            op=mybir.AluOpType.mult,
                        )
                        nc.sync.dma_start(st_sb[64:128, dc, nsl], tmp[:])

                for mc in range(MC):
                    stp = [ps_b.tile([128, 1024], F32, tag="st", name=f"st{i}")
                           for i in range(2)]
                    # hp-outer so each head's exp can start as soon as that
                    # head's two matmuls land (ScalarE gets a head start)
                    for hp in range(2):
                        for hf in range(2):
                            nc.tensor.matmul(
                                stp[hp][:, hf * 512:(hf + 1) * 512],
                                kT_z[:, dc, hp, mc * 128:(mc + 1) * 128],
                                qT_sb[:, dc,
                                      nb2 * 1024 + hf * 512:
                                      nb2 * 1024 + (hf + 1) * 512],
                                start=True, stop=True,
                            )
                        nc.scalar.activation(
                            pT_sb[hp][:, mc % 8, :], stp[hp][:],
                            mybir.ActivationFunctionType.Exp,
                            scale=SCALE,
                        )
                    while emitted < len(bg) * (mc + 1) // MC:
                        bg[emitted]()
                        emitted += 1
                    if mc >= 4:
                        s = mc - 4
                        pv_sub(s // 4, s % 4)
                # tail: last subchunk (mc 12-15), normalizations, tail ops
                pv_sub(3, 0)
                pv_sub(3, 1)
                norm(0)
                norm(1)
                if tail_ops:
                    for em in tail_ops[0]:
                        em()
                pv_sub(3, 2)
                pv_sub(3, 3)
                norm(2)
                norm(3)
                if tail_ops:
                    for em in tail_ops[1]:
                        em()

            def op_group(nck, jb, tail=False):
                # out-projection; wo lives in wq_sb (aliased after last qt
                # use).  Tail groups alternate between the small-psum ring
                # and the (by then idle) big-psum pool for a deeper acc ring.
                if tail and (nck + jb) % 2:
                    acc = ps_b.tile([128, 1024], F32, tag="st", name="st0")
                    acc_ap = acc[:, 0:512]
                else:
                    acc = ps_s.tile([128, 512], F32, tag="po", name="po")
                    acc_ap = acc[:]
                for cc in range(DC):
                    nc.tensor.matmul(
                        acc_ap,
                        st_sb[:, cc, nck * 128:(nck + 1) * 128],
                        wq_sb[:, cc, 4 * jb:4 * jb + 4, :],
                        start=(cc == 0), stop=(cc == DC - 1),
                    )
                ot = wp.tile([128, 512], BF16, tag="out")
                # tail copies go to the (idle-by-then) scalar engine and the
                # final DMAs alternate queues
                if tail:
                    nc.scalar.copy(out=ot[:], in_=acc_ap)
                else:
                    nc.vector.tensor_copy(out=ot[:], in_=acc_ap)
                dq = nc.gpsimd if (nck + jb) % 2 else nc.sync
                dq.dma_start(
                    out.ap()[nck * 128:(nck + 1) * 128,
                             jb * 512:(jb + 1) * 512],
                    ot[:],
                )

            def kt_group(dc, nb, chunk, eng=None):
                eng = eng or nc.vector
                acc = ps_s.tile([128, 512], F32, tag="po")
                for ec in range(EC):
                    nc.tensor.matmul(
                        acc[:],
                        wk_sb[:, dc, ec, :],
                        chunk[:, ec, :],
                        start=(ec == 0), stop=(ec == EC - 1),
                    )
                nsl = slice(nb * 512, (nb + 1) * 512)
                # write the two 64-row head halves into their padded slots
                eng.tensor_scalar_add(
                    kT_z[0:64, dc, 0, nsl], acc[0:64, :],
                    bk_sb[0:64, dc:dc + 1],
                )
                eng.tensor_scalar_add(
                    kT_z[64:128, dc, 1, nsl], acc[64:128, :],
                    bk_sb[64:128, dc:dc + 1],
                )

            def qt_group(dc, nb, chunk, eng=None):
                eng = eng or nc.vector
                acc = ps_s.tile([128, 512], F32, tag="po")
                for ec in range(EC):
                    nc.tensor.matmul(
                        acc[:],
                        wq_sb[:, dc, ec, :],
                        chunk[:, ec, :],
                        start=(ec == 0), stop=(ec == EC - 1),
                    )
                eng.tensor_scalar_add(
                    qT_sb[:, dc, nb * 512:(nb + 1) * 512],
                    acc[:], bq_sb[:, dc:dc + 1],
                )

            def v_group(mc, chunk, eng=None):
                eng = eng or nc.vector
                mi = mc % 4
                accv = ps_s.tile([128, 512], F32, tag="po")
                for ec in range(EC):
                    nc.tensor.matmul(
                        accv[:],
                        chunk[:, ec, mi * 128:(mi + 1) * 128],
                        wv_sb[:, ec, :],
                        start=(ec == 0), stop=(ec == EC - 1),
                    )
                eng.tensor_tensor(
                    out=v_sb[:, mc, :, HD:128],
                    in0=accv[:].rearrange("p (h d) -> p h d", h=HPC),
                    in1=bvb_sb[:].rearrange("p (h d) -> p h d", h=HPC),
                    op=mybir.AluOpType.add,
                )

            def proj_unit(fn, dc, nb, src):
                def emit():
                    fn(dc, nb, load_chunk(src, nb))
                return emit

            def wo_load_unit():
                # overwrite wq_sb with wo once the last qt_group has read it
                def emit():
                    nc.gpsimd.dma_start(wq_sb[:], wo.ap())
                return emit

            # ---- prefix: kT[0] + qT[0], then attention with all other
            # projection / output work paced through the mc loops as
            # background PE groups.
            # prefix drains go on vector: gpsimd is chewing through the init
            # memsets at this point and would delay the kT/qT writes.  The
            # prefix is DMA-bound, so chunk loads alternate between the sync
            # and (idle until the first exp) scalar DMA queues, and only the
            # slices pair (0,0) actually reads are produced here.
            kt_group(0, 0, load_chunk(cT, 0, q=nc.sync), eng=nc.vector)
            qt_group(0, 0, load_chunk(xT, 0, q=nc.scalar), eng=nc.vector)
            qt_group(0, 1, load_chunk(xT, 1, q=nc.sync), eng=nc.vector)

            vcache = {}
            def v_unit2(mc):
                def emit():
                    nb = mc // 4
                    if vcache.get("nb") != nb:
                        vcache["nb"] = nb
                        vcache["c"] = load_chunk(cT, nb)
                    v_group(mc, vcache["c"])
                return emit

            # pair (0,0) consumes kT dc0 m-chunks progressively (nb = mc//4)
            # and v chunks with a 4-slot lag, so the remaining dc0 projections
            # stream as background work.  ORDER MATTERS: Tile dependencies
            # follow emission order, so each unit must be emitted before the
            # first consumer of its output.
            bg00 = [proj_unit(kt_group, 0, 1, cT)]
            bg00 += [v_unit2(mc) for mc in range(8)]
            bg00 += [proj_unit(kt_group, 0, 2, cT),
                     proj_unit(kt_group, 0, 3, cT)]
            bg00 += [v_unit2(mc) for mc in range(8, MC)]
            bg00 += [proj_unit(qt_group, 0, 2, xT),
                     proj_unit(qt_group, 0, 3, xT)]
            bg_plan = {
                (0, 0): bg00,
                (0, 1): [proj_unit(kt_group, 1, nb, cT) for nb in range(NB)]
                      + [proj_unit(qt_group, 1, nb, xT) for nb in range(NB)],
                (1, 0): [proj_unit(kt_group, 2, nb, cT) for nb in range(NB)],
                (1, 1): [proj_unit(qt_group, 2, nb, xT) for nb in range(NB)],
                (2, 0): [proj_unit(kt_group, 3, nb, cT) for nb in range(NB)],
                (2, 1): [proj_unit(qt_group, 3, nb, xT) for nb in range(NB)]
                      + [wo_load_unit()],
                (3, 1): [(lambda nck=nck, jb=jb: op_group(nck, jb))
                         for nck in range(0, 8) for jb in range(2)],
            }
            # last pair: pull half of the remaining out-projection into the
            # pair tail (nck 8-11 only needs the ns=0 st rows, 12-15 ns=1)
            tail_ops_last = (
                [(lambda nck=nck, jb=jb: op_group(nck, jb, tail=True))
                 for nck in range(8, 12) for jb in range(2)],
                [(lambda nck=nck, jb=jb: op_group(nck, jb, tail=True))
                 for nck in range(12, 16) for jb in range(2)],
            )
            for dc in range(DC):
                for nb2 in range(2):
                    attention_pair(
                        dc, nb2, bg_plan.get((dc, nb2), ()),
                        tail_ops=tail_ops_last if (dc, nb2) == (3, 1) else None,
                    )
    nc.compile()
    return nc


_NC_CACHE = None


def _get_nc():
    global _NC_CACHE
    if _NC_CACHE is None:
        _NC_CACHE = _build_nc()
    return _NC_CACHE


def _wqk_layout(W):
    """[512, 1024] -> [128, DC, EC, 128]: element [p,dc,ec,c] = W[dc*128+c,
    ec*128+p] (partition from the contraction axis; q/k proj stationary)."""
    return np.ascontiguousarray(
        np.asarray(W).reshape(DC, 128, EC, 128).transpose(3, 0, 2, 1)
    ).astype(nbf)


def _wo_layout(W):
    """[512, 1024] -> [128, DC, EC, 128]: element [p,cc,ee,c] = W[cc*128+p,
    ee*128+c] (partition from the CW axis; out-proj moving operand)."""
    return np.ascontiguousarray(
        np.asarray(W).reshape(DC, 128, EC, 128).transpose(1, 0, 2, 3)
    ).astype(nbf)


def make_in_maps(x, context, Wq, bq, Wk, bk, Wv, bv, Wo, bo):
    """Host-side sharding: per-core transposed bf16 operand prep."""
    in_maps = []
    for c in range(N_CORES):
        b, hg = c // 2, c % 2
        cs = slice(hg * CW, hg * CW + CW)
        wv_host = np.ascontiguousarray(
            np.asarray(Wv[cs]).reshape(CW, EC, 128).transpose(2, 1, 0)
        ).astype(nbf)
        in_maps.append({
            "xT": np.ascontiguousarray(x[b].T).astype(nbf),
            "cT": np.ascontiguousarray(context[b].T).astype(nbf),
            "wq": _wqk_layout(Wq[cs]),
            "wk": _wqk_layout(Wk[cs]),
            "wv": wv_host,
            "wo": _wo_layout(np.asarray(Wo)[:, cs].T),
            "bq": np.ascontiguousarray(bq[cs]).astype(np.float32),
            "bk": np.ascontiguousarray(bk[cs]).astype(np.float32),
            "bv": np.ascontiguousarray(bv[cs]).astype(np.float32),
        })
    return in_maps


def gather(results, bo):
    """Host-side unshard: sum the two head-group partials per batch, add bo."""
    out = np.empty((B, N, DIM), np.float32)
    for b in range(B):
        out[b] = (results[2 * b]["out"].astype(np.float32)
                  + results[2 * b + 1]["out"].astype(np.float32))
    out += np.asarray(bo, np.float32)[None, None, :]
    return out


def kernel(x, context, Wq, bq, Wk, bk, Wv, bv, Wo, bo):
    nc = _get_nc()
    in_maps = make_in_maps(x, context, Wq, bq, Wk, bk, Wv, bv, Wo, bo)
    res = run_bass_kernel_spmd(nc, in_maps, list(range(N_CORES)))
    return gather(res.results, bo)
